# revision 13
# baseline (speedup 1.0000x reference)
"""Trainium2 Bass kernel for nn_AggFeatureSeqEncoder (histogram binning).

Algorithm (per core, pure data-parallel over B):
  Radix outer-product histogram on the TensorEngine. Each category value v
  is split into digits v = 8*hi + lo. For each row, a 128-long timestep
  chunk is contracted on the PE: stationary = [lo-one-hots(mcc) x
  {1,val,val^2} | hi-one-hots(tr)] (40 cols), moving = [hi-one-hots(mcc) |
  lo-one-hots(tr) x {1,val,val^2}] (56 cols). PSUM accumulates over the 16
  chunks, yielding all cnt/sum/sum-sq bins of both categories for that row
  in one [40, 56] tile. Bins-major results are staged to a DRAM scratch in
  bf16, transpose-loaded back to row-major, and reduced to the final
  count/mean/std/distinct features in fp32.

  Digit tensors are extracted once (row-major, int round trick), cast to
  bf16 and DMA-transposed to timestep-major layout.

Sharding: B=4096 rows split 512/core over 8 cores.
"""

import numpy as np

import concourse.bass as bass
import concourse.bacc as bacc
import concourse.mybir as mybir
from concourse.tile import TileContext
from concourse.bass_utils import run_bass_kernel_spmd

B, T = 4096, 2048
NCORES = 8
R = B // NCORES          # 512 rows per core
C = 16                   # t-chunks of 128; t = 128*c + p after DMA transpose
Q = 128                  # rows per processing quarter (= transpose tile)
G = 16                   # rows per PE group (PSUM: 2 banks)
NGRP = Q // G            # groups per quarter
VM, VT = 256, 128
H = 4 + 3 * VM + 3 * VT + 2  # 1158
NS, NM = 40, 56          # stationary / moving column counts
EPS = 1e-9
DELTA = 1.0 / 128        # relative clamp on the bin variance (kills bf16
                         # residue that the cnt==1 eps would amplify by 1e9)

f32 = mybir.dt.float32
bf16 = mybir.dt.bfloat16
i32 = mybir.dt.int32
i16 = mybir.dt.int16
Alu = mybir.AluOpType
Act = mybir.ActivationFunctionType

# output column layout
O_MCNT, O_MMEAN, O_MSTD = 4, 4 + VM, 4 + 2 * VM
O_TCNT, O_TMEAN, O_TSTD = 4 + 3 * VM, 4 + 3 * VM + VT, 4 + 3 * VM + 2 * VT
O_DIST = H - 2

# ablation knobs (bench/ablation only; production defaults)
OPTS = {"mask_oh": True, "mask_prod": True, "chunks": C}


def _prep_quarter(nc, prep, der, amount, mcc, tr, q):
    """Load + digit-extract + transpose one 128-row quarter; returns the
    timestep-major derived tiles [128, C, Q]."""
    rows = slice(q * Q, (q + 1) * Q)

    m_bf = prep.tile([128, T], bf16, tag="m_bf")
    nc.gpsimd.dma_start(out=m_bf[:], in_=mcc[rows, :])
    t_bf = prep.tile([128, T], bf16, tag="t_bf")
    nc.gpsimd.dma_start(out=t_bf[:], in_=tr[rows, :])
    a_bf = prep.tile([128, T], bf16, tag="a_bf")
    nc.gpsimd.dma_start(out=a_bf[:], in_=amount[rows, :])

    hiM = der.tile([128, C, Q], bf16, tag="hiM")
    loM = der.tile([128, C, Q], bf16, tag="loM")
    hiT = der.tile([128, C, Q], bf16, tag="hiT")
    loT = der.tile([128, C, Q], bf16, tag="loT")

    # hi = floor(code/8) via round((code-3.5)/8) on the f32->int convert
    # (extraction on GpSimd to keep the Vector engine free for mask-gen)
    for src, hid, lod in ((m_bf, hiM, loM), (t_bf, hiT, loT)):
        h_i = prep.tile([128, T], i16, tag="h_i")
        nc.gpsimd.tensor_scalar(h_i[:], src[:], 3.5, 0.125, Alu.subtract, Alu.mult)
        h_b = prep.tile([128, T], bf16, tag="h_b")
        nc.gpsimd.tensor_scalar(h_b[:], h_i[:], 0, None, Alu.add)
        l_b = prep.tile([128, T], bf16, tag="l_b")
        nc.vector.scalar_tensor_tensor(
            l_b[:], h_b[:], -8.0, src[:], Alu.mult, Alu.add
        )
        nc.sync.dma_start(out=hid[:], in_=h_b[:], transpose=True)
        nc.sync.dma_start(out=lod[:], in_=l_b[:], transpose=True)

    aT = prep.tile([128, C, Q], bf16, tag="aT")
    nc.sync.dma_start(out=aT[:], in_=a_bf[:], transpose=True)

    # val = sign(a) * (exp(|a|) - 1), val2 = val^2 (timestep-major)
    sgn = prep.tile([128, C, Q], bf16, tag="sgn")
    nc.scalar.activation(sgn[:], aT[:], Act.Sign)
    ex = prep.tile([128, C, Q], bf16, tag="ex")
    nc.scalar.activation(ex[:], aT[:], Act.Abs)
    nc.scalar.activation(ex[:], ex[:], Act.Exp)
    valT = der.tile([128, C, Q], bf16, tag="valT")
    nc.vector.scalar_tensor_tensor(valT[:], ex[:], 1.0, sgn[:], Alu.subtract, Alu.mult)
    val2T = der.tile([128, C, Q], bf16, tag="val2T")
    nc.scalar.activation(val2T[:], valT[:], Act.Square)
    return hiM, loM, hiT, loT, valT, val2T


def _emit_group(nc, maskp, stgp, psump, drv, scratch3, q, g):
    """Masks + PE + stage for one 16-row group."""
    hiM, loM, hiT, loT, valT, val2T = drv
    rsl = slice(g * G, (g + 1) * G)

    ST = maskp.tile([128, NS, C, G], bf16, tag="ST")
    MT = maskp.tile([128, NM, C, G], bf16, tag="MT")
    do_oh, do_prod = OPTS["mask_oh"], OPTS["mask_prod"]

    # stationary: [0:8]=lo_m oh, [8:16]=lo_m*val, [16:24]=lo_m*val2,
    #             [24:40]=hi_t oh
    for k in range(8):
        if not do_oh: break
        nc.vector.tensor_scalar(ST[:, k], loM[:, :, rsl], float(k), None, Alu.is_equal)
    for k in range(8):
        if not do_prod: break
        nc.vector.scalar_tensor_tensor(
            ST[:, 8 + k], loM[:, :, rsl], float(k), valT[:, :, rsl],
            Alu.is_equal, Alu.mult,
        )
    for k in range(8):
        if not do_prod: break
        nc.vector.scalar_tensor_tensor(
            ST[:, 16 + k], loM[:, :, rsl], float(k), val2T[:, :, rsl],
            Alu.is_equal, Alu.mult,
        )
    for j in range(16):
        if not do_oh: break
        nc.vector.tensor_scalar(
            ST[:, 24 + j], hiT[:, :, rsl], float(j), None, Alu.is_equal
        )
    # moving: [0:32]=hi_m oh, [32:40]=lo_t oh, [40:48]=lo_t*val,
    #         [48:56]=lo_t*val2
    for j in range(32):
        if not do_oh: break
        nc.vector.tensor_scalar(MT[:, j], hiM[:, :, rsl], float(j), None, Alu.is_equal)
    for k in range(8):
        if not do_oh: break
        nc.vector.tensor_scalar(
            MT[:, 32 + k], loT[:, :, rsl], float(k), None, Alu.is_equal
        )
    for k in range(8):
        if not do_prod: break
        nc.vector.scalar_tensor_tensor(
            MT[:, 40 + k], loT[:, :, rsl], float(k), valT[:, :, rsl],
            Alu.is_equal, Alu.mult,
        )
    for k in range(8):
        if not do_prod: break
        nc.vector.scalar_tensor_tensor(
            MT[:, 48 + k], loT[:, :, rsl], float(k), val2T[:, :, rsl],
            Alu.is_equal, Alu.mult,
        )

    PS = psump.tile([NS, 1024], f32, tag="PS")  # 2 banks: rows 0:9 / 9:16
    for r in range(G):
        off = r * NM if r < 9 else 512 + (r - 9) * NM
        nch = OPTS["chunks"]
        for c in range(nch):
            nc.tensor.matmul(
                PS[:, off : off + NM],
                ST[:, :, c, r],
                MT[:, :, c, r],
                start=(c == 0),
                stop=(c == nch - 1),
            )

    SG = stgp.tile([NS, NM, G], bf16, tag="SG")
    nc.scalar.copy(
        SG[:, :, 0:9], PS[:, 0 : 9 * NM].rearrange("p (r m) -> p m r", r=9)
    )
    nc.scalar.copy(
        SG[:, :, 9:16], PS[:, 512 : 512 + 7 * NM].rearrange("p (r m) -> p m r", r=7)
    )
    off = q * Q + g * G
    nc.sync.dma_start(out=scratch3[:, :, off : off + G], in_=SG[:])


def _postproc_block(nc, postp, scratch2, sl_sb, out, rb):
    """Transpose-load one 128-row block of raw bins and derive features."""
    TRB = postp.tile([128, NS * NM], bf16, tag="TRB")
    nc.sync.dma_start(
        out=TRB[:], in_=scratch2[:, rb * Q : (rb + 1) * Q], transpose=True
    )
    T3 = TRB[:].rearrange("p (s m) -> p s m", s=NS)

    out_sb = postp.tile([128, H], f32, tag="osb")

    # raw bins -> f32 (v = 8*hi + lo); copies on ScalarE to spare the DVE
    sv = postp.tile([128, VM + VT], f32, tag="sv")
    sv2 = postp.tile([128, VM + VT], f32, tag="sv2")
    nc.scalar.copy(
        out_sb[:, O_MCNT : O_MCNT + VM].rearrange("p (h l) -> p h l", h=32),
        T3[:, 0:8, 0:32].rearrange("p l h -> p h l"),
    )
    nc.scalar.copy(out_sb[:, O_TCNT : O_TCNT + VT], T3[:, 24:40, 32:40])
    nc.scalar.copy(
        sv[:, 0:VM].rearrange("p (h l) -> p h l", h=32),
        T3[:, 8:16, 0:32].rearrange("p l h -> p h l"),
    )
    nc.scalar.copy(sv[:, VM:], T3[:, 24:40, 40:48])
    nc.scalar.copy(
        sv2[:, 0:VM].rearrange("p (h l) -> p h l", h=32),
        T3[:, 16:24, 0:32].rearrange("p l h -> p h l"),
    )
    nc.scalar.copy(sv2[:, VM:], T3[:, 24:40, 48:56])

    # per-category features
    scr = postp.tile([128, VM + VT], f32, tag="scr")
    rc = postp.tile([128, VM + VT], f32, tag="rc")
    for cat, (o_cnt, o_mean, o_std, V, c0) in enumerate(
        ((O_MCNT, O_MMEAN, O_MSTD, VM, 0), (O_TCNT, O_TMEAN, O_TSTD, VT, VM))
    ):
        cnt = out_sb[:, o_cnt : o_cnt + V]
        svc = sv[:, c0 : c0 + V]
        sv2c = sv2[:, c0 : c0 + V]
        s = scr[:, c0 : c0 + V]
        r = rc[:, c0 : c0 + V]
        nc.vector.tensor_scalar_add(s, cnt, EPS)
        nc.vector.reciprocal(r, s)
        mean = out_sb[:, o_mean : o_mean + V]
        nc.vector.tensor_tensor(mean, svc, r, Alu.mult)
        # av = clip(sv2*(1-delta) - sv*mean, 0)
        nc.vector.tensor_tensor(s, svc, mean, Alu.mult)
        nc.vector.scalar_tensor_tensor(
            s, sv2c, 1.0 - DELTA, s, Alu.mult, Alu.subtract
        )
        nc.vector.tensor_scalar_max(s, s, 0.0)
        nc.vector.tensor_scalar(r, cnt, 1.0, 0.0, Alu.subtract, Alu.max)
        nc.vector.tensor_scalar_add(r, r, EPS)
        nc.vector.reciprocal(r, r)
        nc.vector.tensor_tensor(s, s, r, Alu.mult)
        nc.scalar.activation(out_sb[:, o_std : o_std + V], s, Act.Sqrt)
        # distinct count
        nc.vector.tensor_scalar(
            s, cnt, 0.0, None, Alu.is_gt, Alu.add,
            accum_out=out_sb[:, O_DIST + cat : O_DIST + cat + 1],
        )

    # row stats from the mcc bins (sum over bins == sum over t)
    sumv = postp.tile([128, 1], f32, tag="sumv")
    sumv2 = postp.tile([128, 1], f32, tag="sumv2")
    nc.vector.tensor_scalar(
        scr[:, 0:VM], sv[:, 0:VM], 0.0, None, Alu.add, Alu.add, accum_out=sumv[:]
    )
    nc.vector.tensor_scalar(
        scr[:, 0:VM], sv2[:, 0:VM], 0.0, None, Alu.add, Alu.add, accum_out=sumv2[:]
    )
    sl_f = postp.tile([128, 1], f32, tag="slf")
    nc.vector.tensor_scalar_add(sl_f[:], sl_sb[:, rb : rb + 1], 0)
    t0 = postp.tile([128, 1], f32, tag="t0")
    r1 = postp.tile([128, 1], f32, tag="r1")
    nc.vector.tensor_scalar_add(t0[:], sl_f[:], EPS)
    nc.vector.reciprocal(r1[:], t0[:])
    nc.vector.tensor_scalar_add(out_sb[:, 0:1], sl_f[:], 0.0)
    nc.vector.tensor_scalar_add(out_sb[:, 1:2], sumv[:], 0.0)
    nc.vector.tensor_tensor(out_sb[:, 2:3], sumv[:], r1[:], Alu.mult)
    nc.vector.tensor_tensor(t0[:], sumv[:], r1[:], Alu.mult)
    nc.vector.tensor_tensor(t0[:], sumv[:], t0[:], Alu.mult)
    nc.vector.tensor_tensor(t0[:], sumv2[:], t0[:], Alu.subtract)
    nc.vector.tensor_scalar_max(t0[:], t0[:], 0.0)
    d = postp.tile([128, 1], f32, tag="d")
    nc.vector.tensor_scalar(d[:], sl_f[:], 1.0, 0.0, Alu.subtract, Alu.max)
    nc.vector.tensor_scalar_add(d[:], d[:], EPS)
    nc.vector.reciprocal(d[:], d[:])
    nc.vector.tensor_tensor(t0[:], t0[:], d[:], Alu.mult)
    nc.scalar.activation(out_sb[:, 3:4], t0[:], Act.Sqrt)

    nc.gpsimd.dma_start(out=out[rb * Q : (rb + 1) * Q, :], in_=out_sb[:])


def build_nc(reps=None):
    """reps: if set, wrap the whole pipeline in a hardware loop executing it
    `reps` times — used only for slope-based timing (bench), not grading."""
    nc = bacc.Bacc()
    amount = nc.declare_dram_parameter("amount", [R, T], f32, False)
    mcc = nc.declare_dram_parameter("mcc", [R, T], i32, False)
    tr = nc.declare_dram_parameter("tr", [R, T], i32, False)
    seq = nc.declare_dram_parameter("seq", [R, 1], i32, False)
    out = nc.declare_dram_parameter("out", [R, H], f32, True)

    with TileContext(nc) as tc:
        with (
            tc.tile_pool(name="prep", bufs=1) as prep,
            tc.tile_pool(name="der", bufs=2) as der,
            tc.tile_pool(name="mask", bufs=2) as maskp,
            tc.tile_pool(name="stg", bufs=2) as stgp,
            tc.tile_pool(name="post", bufs=1) as postp,
            tc.tile_pool(name="psum", bufs=2, space="PSUM") as psump,
            tc.tile_pool(name="dram", bufs=1, space="DRAM") as dramp,
        ):
            scratch = dramp.tile([NS, NM, R], bf16, tag="scratch")
            scratch3 = scratch[:]
            scratch2 = scratch[:].rearrange("a b r -> (a b) r")

            def body():
                sl_sb = der.tile([128, 4], i32, tag="sl")
                nc.sync.dma_start(
                    out=sl_sb[:],
                    in_=seq[:].rearrange("(a p) one -> p (a one)", p=128),
                )
                for q in range(R // Q):
                    drv = _prep_quarter(nc, prep, der, amount, mcc, tr, q)
                    for g in range(NGRP):
                        _emit_group(nc, maskp, stgp, psump, drv, scratch3, q, g)
                    _postproc_block(nc, postp, scratch2, sl_sb, out, q)

            if reps is None:
                body()
            else:
                with tc.For_i(0, reps, 1):
                    body()
    return nc


_NC = None


def _make_in_maps(inputs):
    amount, mcc_code = inputs["amount"], inputs["mcc_code"]
    tr_type, seq_lens = inputs["tr_type"], inputs["seq_lens"]
    in_maps = []
    for c in range(NCORES):
        rs = slice(c * R, (c + 1) * R)
        in_maps.append(
            {
                "amount": np.ascontiguousarray(amount[rs]),
                "mcc": np.ascontiguousarray(mcc_code[rs]),
                "tr": np.ascontiguousarray(tr_type[rs]),
                "seq": np.ascontiguousarray(seq_lens[rs]).reshape(R, 1),
            }
        )
    return in_maps


def kernel(amount, mcc_code, tr_type, seq_lens):
    global _NC
    if _NC is None:
        _NC = build_nc()
        _NC.finalize()
    in_maps = _make_in_maps(
        {
            "amount": amount,
            "mcc_code": mcc_code,
            "tr_type": tr_type,
            "seq_lens": seq_lens,
        }
    )
    res = run_bass_kernel_spmd(_NC, in_maps, list(range(NCORES))).results
    return np.concatenate([res[c]["out"] for c in range(NCORES)], axis=0)
